# revision 6
# baseline (speedup 1.0000x reference)
"""Trainium2 Bass kernel for the CurvedAssociativeMemory fixed-point iteration.

Computes, for `steps` iterations:
    s <- sign(s @ (J + J^T) + h + kappa * softmax(s, axis=-1))

Strategy: data-parallel over the batch dim across 8 NeuronCores (512 rows
per core), J replicated and streamed from HBM each step.

The matmul runs as an error-corrected 16-bit split, host-precomputed:
    Jhi = fp16(J)   (flush-to-zero below 2^-14 so host and PE agree)
    Jlo = bf16(J - Jhi)
so J is carried to ~23 mantissa bits (11 fp16 + residual in bf16's 8,
shifted down 2^-12).  After step 1 the state is exactly {-1, 0, +1}
(sign outputs), so +-1 x Jhi / +-1 x Jlo products are exact and two
passes (fp16 then bf16, accumulated in one fp32 PSUM group) match native
fp32 to sum-order rounding (~1e-7) — far inside this iteration's
sign-flip noise floor.  Step 1's Gaussian state is split the same way
(chi = fp16(c), clo = bf16(c - chi)) and runs three passes:
chi x Jhi  +  bf16(c) x Jlo  +  clo x bf16(J); the neglected cross terms
are ~2^-21 relative.  Measured on the real inputs the whole scheme costs
a few hundred sign flips out of 16.7M (rel err < 1e-2, tolerance 2e-2).

16-bit operands also make the PE stream cheap: weight loads are 2-byte
(fast-weight-load eligible, hidden behind the 512-element moving pass)
and no on-chip fp32r rounding passes are needed, keeping the DVE free
for softmax/epilogue work.

The softmax term is kappa * softmax ~ 1e-4 of the matmul scale, so it
only needs fp32 elementwise accuracy: max-subtract, ACT-table exp,
chunked free-dim reduce_sum, DVE reciprocal folded with kappa.
"""

import numpy as np

N = 4096          # feature dim
B = 4096          # total batch
N_CORES = 8
B_SH = B // N_CORES   # 512 batch rows per core
P = 128               # partitions
NCHUNK = 512          # matmul moving free-dim per chunk (1 PSUM bank)
KO = N // P           # 32 k-tiles
NO = N // NCHUNK      # 8 n-chunks
BT = B_SH // P        # 4 batch tiles per core

# tuning knobs (overridable before _build for experiments)
REPEAT = 1  # timing only: run the whole step body REPEAT times via a HW loop
JPOOL_BUFS = 3
SCRATCH_BUFS = 2
PSUM_BUFS = 8


def _build(steps: int, kappa: float, has_h: bool):
    import concourse.bass as bass
    import concourse.tile as tile
    import concourse.mybir as mybir
    from concourse import bacc
    from concourse.masks import make_identity

    F32 = mybir.dt.float32
    F16 = mybir.dt.float16
    BF16 = mybir.dt.bfloat16
    AF = mybir.ActivationFunctionType

    nc = bacc.Bacc(None)
    s_in = nc.dram_tensor("s", [B_SH, N], F32, kind="ExternalInput")
    jhi_in = nc.dram_tensor("jhi", [N, N], F16, kind="ExternalInput")
    jlo_in = nc.dram_tensor("jlo", [N, N], BF16, kind="ExternalInput")
    j1b_in = nc.dram_tensor("j1b", [N, N], BF16, kind="ExternalInput")
    h_in = nc.dram_tensor("h", [N], F32, kind="ExternalInput") if has_h else None
    out = nc.dram_tensor("out", [B_SH, N], F32, kind="ExternalOutput")

    with tile.TileContext(nc) as tc:
        with (
            tc.tile_pool(name="persist", bufs=1) as persist,
            tc.tile_pool(name="jpool", bufs=JPOOL_BUFS) as jpool,
            tc.tile_pool(name="scratch", bufs=SCRATCH_BUFS) as scratch,
            tc.tile_pool(name="stats", bufs=1) as stats,
            tc.tile_pool(name="psum", bufs=PSUM_BUFS, space="PSUM") as psum,
        ):
            ident = persist.tile([P, P], F32, tag="ident", name="ident")
            make_identity(nc, ident)

            # persistent state: c in natural layout, 4 tiles of [128, N]
            c = [persist.tile([P, N], F32, tag=f"c{bt}", name=f"c{bt}") for bt in range(BT)]
            for bt in range(BT):
                nc.sync.dma_start(out=c[bt], in_=s_in.ap()[bt * P:(bt + 1) * P, :])

            # transposed state: fp16 (hi) + bf16 copies, plus step-1 residual
            cTh = [persist.tile([P, B_SH], F16, tag=f"th{k}", name=f"th{k}") for k in range(KO)]
            cTb = [persist.tile([P, B_SH], BF16, tag=f"tb{k}", name=f"tb{k}") for k in range(KO)]
            cTl = [persist.tile([P, B_SH], BF16, tag=f"tl{k}", name=f"tl{k}") for k in range(KO)]

            h_bc = None
            if has_h:
                h_bc = persist.tile([P, N], F32, tag="hb", name="hb")
                h_ap = h_in.ap()
                nc.sync.dma_start(
                    out=h_bc,
                    in_=bass.AP(tensor=h_ap.tensor, offset=h_ap.offset,
                                ap=[[0, P], [1, N]]),
                )

            mx = [stats.tile([P, 1], F32, tag=f"mx{bt}", name=f"mx{bt}") for bt in range(BT)]
            krS = [stats.tile([P, 1], F32, tag=f"kr{bt}", name=f"kr{bt}") for bt in range(BT)]

            def softmax_stats(bt):
                """mx[bt], krS[bt] = rowmax(c[bt]), kappa / sum(exp(c - mx))."""
                nc.vector.reduce_max(out=mx[bt], in_=c[bt],
                                     axis=mybir.AxisListType.X)
                acc = stats.tile([P, 1], F32, tag="acc", name="acc")
                for n in range(NO):
                    nsl = slice(n * NCHUNK, (n + 1) * NCHUNK)
                    ech = scratch.tile([P, NCHUNK], F32, tag="ech", name="ech")
                    nc.vector.tensor_scalar_sub(out=ech, in0=c[bt][:, nsl],
                                                scalar1=mx[bt])
                    nc.scalar.activation(out=ech, in_=ech, func=AF.Exp)
                    ssum = stats.tile([P, 1], F32, tag="ssum", name="ssum")
                    nc.vector.reduce_sum(out=ssum, in_=ech,
                                         axis=mybir.AxisListType.X)
                    if n == 0:
                        nc.vector.tensor_copy(out=acc, in_=ssum)
                    else:
                        nc.vector.tensor_add(out=acc, in0=acc, in1=ssum)
                nc.vector.reciprocal(out=krS[bt], in_=acc)
                nc.scalar.mul(out=krS[bt], in_=krS[bt], mul=float(kappa))

            def transpose_state(k, first):
                """PE-transpose all 4 batch tiles of c's k-slice into one PSUM
                tile; cast to cTh (fp16) and cTb (bf16); on step 1 also write
                the fp16 residual into cTl (bf16)."""
                ps_t = psum.tile([P, NCHUNK], F32, tag="pb", name="ps_t")
                for bt in range(BT):
                    nc.tensor.transpose(ps_t[:, bt * P:(bt + 1) * P],
                                        c[bt][:, k * P:(k + 1) * P], ident)
                nc.vector.tensor_copy(out=cTh[k], in_=ps_t)
                nc.vector.tensor_copy(out=cTb[k], in_=ps_t)
                if first:
                    nc.vector.tensor_sub(out=cTl[k], in0=ps_t, in1=cTh[k])

            def matmul_sweep(first_step, last_step):
                """Phase B over all n-chunks.

                steps>=2: 2 passes  (cTh x Jhi fp16, cTb x Jlo bf16)
                step 1:   3 passes  (+ cTl x bf16(J))
                On the last step each finished chunk streams straight to DRAM.
                """
                for n in range(NO):
                    nsl = slice(n * NCHUNK, (n + 1) * NCHUNK)
                    pm_t = [psum.tile([P, NCHUNK], F32, tag="pb", name="pm")
                            for _ in range(BT)]
                    for k in range(KO):
                        ksl = slice(k * P, (k + 1) * P)
                        jh = jpool.tile([P, NCHUNK], F16, tag="jh", name="jh")
                        nc.sync.dma_start(out=jh, in_=jhi_in.ap()[ksl, nsl])
                        jl = jpool.tile([P, NCHUNK], BF16, tag="jl", name="jl")
                        nc.sync.dma_start(out=jl, in_=jlo_in.ap()[ksl, nsl])
                        if first_step:
                            j1 = jpool.tile([P, NCHUNK], BF16, tag="j1", name="j1")
                            nc.sync.dma_start(out=j1, in_=j1b_in.ap()[ksl, nsl])
                        last_k = k == KO - 1
                        for bt in range(BT):
                            bsl = slice(bt * P, (bt + 1) * P)
                            nc.tensor.matmul(pm_t[bt], cTh[k][:, bsl], jh,
                                             start=(k == 0), stop=False)
                            nc.tensor.matmul(pm_t[bt], cTb[k][:, bsl], jl,
                                             start=False,
                                             stop=last_k and not first_step)
                            if first_step:
                                nc.tensor.matmul(pm_t[bt], cTl[k][:, bsl], j1,
                                                 start=False, stop=last_k)
                    for bt in range(BT):
                        m_sl = pm_t[bt]
                        u = None
                        if has_h:
                            u = scratch.tile([P, NCHUNK], F32, tag="u", name="u")
                            nc.vector.tensor_add(out=u, in0=m_sl, in1=h_bc[:, nsl])
                        q = scratch.tile([P, NCHUNK], F32, tag="q", name="q")
                        nc.vector.tensor_scalar_sub(out=q, in0=c[bt][:, nsl],
                                                    scalar1=mx[bt])
                        nc.scalar.activation(out=q, in_=q, func=AF.Exp)
                        nc.vector.tensor_scalar_mul(out=q, in0=q, scalar1=krS[bt])
                        u2 = scratch.tile([P, NCHUNK], F32, tag="u2", name="u2")
                        nc.vector.tensor_add(out=u2, in0=u if has_h else m_sl, in1=q)
                        nc.scalar.activation(out=c[bt][:, nsl], in_=u2, func=AF.Sign)
                        if last_step:
                            nc.sync.dma_start(
                                out=out.ap()[bt * P:(bt + 1) * P, nsl],
                                in_=c[bt][:, nsl])

            def emit_steps(first_is_gaussian=True, stream_out=True):
                for _step in range(steps):
                    first = first_is_gaussian and _step == 0
                    for k in range(KO):
                        transpose_state(k, first)
                    for bt in range(BT):
                        softmax_stats(bt)
                    matmul_sweep(first, stream_out and _step == steps - 1)

            if REPEAT > 1:
                with tc.For_i(0, REPEAT, 1):
                    emit_steps(first_is_gaussian=False, stream_out=False)
                for bt in range(BT):
                    nc.sync.dma_start(out=out.ap()[bt * P:(bt + 1) * P, :], in_=c[bt])
            else:
                emit_steps()

    nc.finalize()
    return nc


LAST_RESULTS = None  # BassKernelResults from the most recent kernel() call
LAST_NC = None       # finalized Bass module from the most recent kernel() call


def kernel(s, J, h, kappa, steps):
    import os
    import ml_dtypes
    from concourse.bass_utils import run_bass_kernel_spmd

    s = np.ascontiguousarray(np.asarray(s, dtype=np.float32))
    J = np.asarray(J, dtype=np.float32)
    h = np.asarray(h, dtype=np.float32)
    kappa_f = float(np.asarray(kappa))
    steps_i = int(np.asarray(steps))

    Jsym = np.ascontiguousarray(J + J.T)
    has_h = bool(np.any(h))

    # host-side 16-bit split of J (see module docstring)
    ftz = np.abs(Jsym) >= np.float32(2.0 ** -14)
    Jhi16 = np.ascontiguousarray((Jsym * ftz).astype(np.float16))
    Jlo16 = np.ascontiguousarray(
        (Jsym.astype(np.float64) - Jhi16.astype(np.float64))
        .astype(np.float32).astype(ml_dtypes.bfloat16))
    J1b = np.ascontiguousarray(Jsym.astype(ml_dtypes.bfloat16))

    nc = _build(steps_i, kappa_f, has_h)
    global LAST_NC
    LAST_NC = nc

    in_maps = []
    for i in range(N_CORES):
        m = {"s": np.ascontiguousarray(s[i * B_SH:(i + 1) * B_SH]),
             "jhi": Jhi16, "jlo": Jlo16, "j1b": J1b}
        if has_h:
            m["h"] = h
        in_maps.append(m)

    trace = os.environ.get("CAM_TRACE", "") == "1"
    res = run_bass_kernel_spmd(nc, in_maps, core_ids=list(range(N_CORES)),
                               trace=trace)
    global LAST_RESULTS
    LAST_RESULTS = res
    out = np.concatenate([r["out"] for r in res.results], axis=0)
    return out.astype(np.float32, copy=False)


if __name__ == "__main__":
    rng = np.random.default_rng(0)
    s = rng.standard_normal((B, N)).astype(np.float32)
    J0 = (0.01 * rng.standard_normal((N, N))).astype(np.float32)
    J = ((J0 + J0.T) / 2).astype(np.float32)
    out = kernel(s=s, J=J, h=np.zeros(N, np.float32),
                 kappa=np.float32(0.2), steps=3)
    print(out.shape, np.unique(out, return_counts=True))


# revision 8
# speedup vs baseline: 1.0301x; 1.0301x over previous
"""Trainium2 Bass kernel for the CurvedAssociativeMemory fixed-point iteration.

Computes, for `steps` iterations:
    s <- sign(s @ (J + J^T) + h + kappa * softmax(s, axis=-1))

Strategy: data-parallel over the batch dim across 8 NeuronCores (512 rows
per core), J replicated and streamed from HBM each step.

The matmul runs as an error-corrected 16-bit split, host-precomputed:
    Jhi = fp16(J)   (flush-to-zero below 2^-14 so host and PE agree)
    Jlo = bf16(J - Jhi)
so J is carried to ~23 mantissa bits (11 fp16 + residual in bf16's 8,
shifted down 2^-12).  After step 1 the state is exactly {-1, 0, +1}
(sign outputs), so +-1 x Jhi / +-1 x Jlo products are exact and two
passes (fp16 then bf16, accumulated in one fp32 PSUM group) match native
fp32 to sum-order rounding (~1e-7) — far inside this iteration's
sign-flip noise floor.  Step 1's Gaussian state is split the same way
(chi = fp16(c), clo = bf16(c - chi)) and runs three passes:
chi x Jhi  +  bf16(c) x Jlo  +  clo x bf16(J); the neglected cross terms
are ~2^-21 relative.  Measured on the real inputs the whole scheme costs
a few hundred sign flips out of 16.7M (rel err < 1e-2, tolerance 2e-2).

16-bit operands also make the PE stream cheap: weight loads are 2-byte
(fast-weight-load eligible, hidden behind the 512-element moving pass)
and no on-chip fp32r rounding passes are needed, keeping the DVE free
for softmax/epilogue work.

The softmax term is kappa * softmax ~ 1e-4 of the matmul scale, so it
only needs fp32 elementwise accuracy: max-subtract, ACT-table exp,
chunked free-dim reduce_sum, DVE reciprocal folded with kappa.
"""

import numpy as np

N = 4096          # feature dim
B = 4096          # total batch
N_CORES = 8
B_SH = B // N_CORES   # 512 batch rows per core
P = 128               # partitions
NCHUNK = 512          # matmul moving free-dim per chunk (1 PSUM bank)
KO = N // P           # 32 k-tiles
NO = N // NCHUNK      # 8 n-chunks
BT = B_SH // P        # 4 batch tiles per core

# tuning knobs (overridable before _build for experiments)
REPEAT = 1  # timing only: run the whole step body REPEAT times via a HW loop
JPOOL_BUFS = 4
SCRATCH_BUFS = 2
PSUM_BUFS = 8


def _build(steps: int, kappa: float, has_h: bool):
    import concourse.bass as bass
    import concourse.tile as tile
    import concourse.mybir as mybir
    from concourse import bacc
    from concourse.masks import make_identity

    F32 = mybir.dt.float32
    F16 = mybir.dt.float16
    BF16 = mybir.dt.bfloat16
    AF = mybir.ActivationFunctionType

    nc = bacc.Bacc(None)
    s_in = nc.dram_tensor("s", [B_SH, N], F32, kind="ExternalInput")
    jhi_in = nc.dram_tensor("jhi", [N, N], F16, kind="ExternalInput")
    jlo_in = nc.dram_tensor("jlo", [N, N], BF16, kind="ExternalInput")
    j1b_in = nc.dram_tensor("j1b", [N, N], BF16, kind="ExternalInput")
    h_in = nc.dram_tensor("h", [N], F32, kind="ExternalInput") if has_h else None
    out = nc.dram_tensor("out", [B_SH, N], F32, kind="ExternalOutput")

    with tile.TileContext(nc) as tc:
        with (
            tc.tile_pool(name="persist", bufs=1) as persist,
            tc.tile_pool(name="jpool", bufs=JPOOL_BUFS) as jpool,
            tc.tile_pool(name="scratch", bufs=SCRATCH_BUFS) as scratch,
            tc.tile_pool(name="stats", bufs=1) as stats,
            tc.tile_pool(name="psum", bufs=PSUM_BUFS, space="PSUM") as psum,
        ):
            ident = persist.tile([P, P], F32, tag="ident", name="ident")
            make_identity(nc, ident)

            # persistent state: c in natural layout, 4 tiles of [128, N]
            c = [persist.tile([P, N], F32, tag=f"c{bt}", name=f"c{bt}") for bt in range(BT)]
            for bt in range(BT):
                nc.sync.dma_start(out=c[bt], in_=s_in.ap()[bt * P:(bt + 1) * P, :])

            # transposed state: fp16 (hi) + bf16 copies, plus step-1 residual
            cTh = [persist.tile([P, B_SH], F16, tag=f"th{k}", name=f"th{k}") for k in range(KO)]
            cTb = [persist.tile([P, B_SH], BF16, tag=f"tb{k}", name=f"tb{k}") for k in range(KO)]
            cTl = [persist.tile([P, B_SH], BF16, tag=f"tl{k}", name=f"tl{k}") for k in range(KO)]

            h_bc = None
            if has_h:
                h_bc = persist.tile([P, N], F32, tag="hb", name="hb")
                h_ap = h_in.ap()
                nc.sync.dma_start(
                    out=h_bc,
                    in_=bass.AP(tensor=h_ap.tensor, offset=h_ap.offset,
                                ap=[[0, P], [1, N]]),
                )

            mx = [stats.tile([P, 1], F32, tag=f"mx{bt}", name=f"mx{bt}") for bt in range(BT)]
            krS = [stats.tile([P, 1], F32, tag=f"kr{bt}", name=f"kr{bt}") for bt in range(BT)]

            def softmax_stats(bt):
                """mx[bt], krS[bt] = rowmax(c[bt]), kappa / sum(exp(c - mx))."""
                nc.vector.reduce_max(out=mx[bt], in_=c[bt],
                                     axis=mybir.AxisListType.X)
                acc = stats.tile([P, 1], F32, tag="acc", name="acc")
                for n in range(NO):
                    nsl = slice(n * NCHUNK, (n + 1) * NCHUNK)
                    ech = scratch.tile([P, NCHUNK], F32, tag="ech", name="ech")
                    nc.vector.tensor_scalar_sub(out=ech, in0=c[bt][:, nsl],
                                                scalar1=mx[bt])
                    nc.scalar.activation(out=ech, in_=ech, func=AF.Exp)
                    ssum = stats.tile([P, 1], F32, tag="ssum", name="ssum")
                    nc.vector.reduce_sum(out=ssum, in_=ech,
                                         axis=mybir.AxisListType.X)
                    if n == 0:
                        nc.vector.tensor_copy(out=acc, in_=ssum)
                    else:
                        nc.vector.tensor_add(out=acc, in0=acc, in1=ssum)
                nc.vector.reciprocal(out=krS[bt], in_=acc)
                nc.scalar.mul(out=krS[bt], in_=krS[bt], mul=float(kappa))

            def transpose_state(k, first):
                """PE-transpose all 4 batch tiles of c's k-slice into one PSUM
                tile; cast to cTh (fp16) and cTb (bf16); on step 1 also write
                the fp16 residual into cTl (bf16)."""
                ps_t = psum.tile([P, NCHUNK], F32, tag="pb", name="ps_t")
                for bt in range(BT):
                    nc.tensor.transpose(ps_t[:, bt * P:(bt + 1) * P],
                                        c[bt][:, k * P:(k + 1) * P], ident)
                nc.vector.tensor_copy(out=cTh[k], in_=ps_t)
                nc.vector.tensor_copy(out=cTb[k], in_=ps_t)
                if first:
                    nc.vector.tensor_sub(out=cTl[k], in0=ps_t, in1=cTh[k])

            def matmul_sweep(first_step, last_step):
                """Phase B over all n-chunks.

                steps>=2: 2 passes  (cTh x Jhi fp16, cTb x Jlo bf16)
                step 1:   3 passes  (+ cTl x bf16(J))
                On the last step each finished chunk streams straight to DRAM.
                """
                for n in range(NO):
                    nsl = slice(n * NCHUNK, (n + 1) * NCHUNK)
                    pm_t = [psum.tile([P, NCHUNK], F32, tag="pb", name="pm")
                            for _ in range(BT)]
                    for k in range(KO):
                        ksl = slice(k * P, (k + 1) * P)
                        # split DMA issues across the two hardware DGE queues
                        # (SP and Activation) so neither serializes the stream
                        jh = jpool.tile([P, NCHUNK], F16, tag="jh", name="jh")
                        nc.sync.dma_start(out=jh, in_=jhi_in.ap()[ksl, nsl])
                        jl = jpool.tile([P, NCHUNK], BF16, tag="jl", name="jl")
                        nc.scalar.dma_start(out=jl, in_=jlo_in.ap()[ksl, nsl])
                        if first_step:
                            j1 = jpool.tile([P, NCHUNK], BF16, tag="j1", name="j1")
                            nc.scalar.dma_start(out=j1, in_=j1b_in.ap()[ksl, nsl])
                        last_k = k == KO - 1
                        for bt in range(BT):
                            bsl = slice(bt * P, (bt + 1) * P)
                            nc.tensor.matmul(pm_t[bt], cTh[k][:, bsl], jh,
                                             start=(k == 0), stop=False)
                            nc.tensor.matmul(pm_t[bt], cTb[k][:, bsl], jl,
                                             start=False,
                                             stop=last_k and not first_step)
                            if first_step:
                                nc.tensor.matmul(pm_t[bt], cTl[k][:, bsl], j1,
                                                 start=False, stop=last_k)
                    for bt in range(BT):
                        m_sl = pm_t[bt]
                        u = None
                        if has_h:
                            u = scratch.tile([P, NCHUNK], F32, tag="u", name="u")
                            nc.vector.tensor_add(out=u, in0=m_sl, in1=h_bc[:, nsl])
                        q = scratch.tile([P, NCHUNK], F32, tag="q", name="q")
                        nc.vector.tensor_scalar_sub(out=q, in0=c[bt][:, nsl],
                                                    scalar1=mx[bt])
                        nc.scalar.activation(out=q, in_=q, func=AF.Exp)
                        nc.vector.tensor_scalar_mul(out=q, in0=q, scalar1=krS[bt])
                        u2 = scratch.tile([P, NCHUNK], F32, tag="u2", name="u2")
                        nc.vector.tensor_add(out=u2, in0=u if has_h else m_sl, in1=q)
                        nc.scalar.activation(out=c[bt][:, nsl], in_=u2, func=AF.Sign)
                        if last_step:
                            nc.sync.dma_start(
                                out=out.ap()[bt * P:(bt + 1) * P, nsl],
                                in_=c[bt][:, nsl])

            def emit_steps(first_is_gaussian=True, stream_out=True):
                for _step in range(steps):
                    first = first_is_gaussian and _step == 0
                    for k in range(KO):
                        transpose_state(k, first)
                    for bt in range(BT):
                        softmax_stats(bt)
                    matmul_sweep(first, stream_out and _step == steps - 1)

            if REPEAT > 1:
                with tc.For_i(0, REPEAT, 1):
                    emit_steps(first_is_gaussian=False, stream_out=False)
                for bt in range(BT):
                    nc.sync.dma_start(out=out.ap()[bt * P:(bt + 1) * P, :], in_=c[bt])
            else:
                emit_steps()

    nc.finalize()
    return nc


LAST_RESULTS = None  # BassKernelResults from the most recent kernel() call
LAST_NC = None       # finalized Bass module from the most recent kernel() call


def kernel(s, J, h, kappa, steps):
    import os
    import ml_dtypes
    from concourse.bass_utils import run_bass_kernel_spmd

    s = np.ascontiguousarray(np.asarray(s, dtype=np.float32))
    J = np.asarray(J, dtype=np.float32)
    h = np.asarray(h, dtype=np.float32)
    kappa_f = float(np.asarray(kappa))
    steps_i = int(np.asarray(steps))

    Jsym = np.ascontiguousarray(J + J.T)
    has_h = bool(np.any(h))

    # host-side 16-bit split of J (see module docstring)
    ftz = np.abs(Jsym) >= np.float32(2.0 ** -14)
    Jhi16 = np.ascontiguousarray((Jsym * ftz).astype(np.float16))
    Jlo16 = np.ascontiguousarray(
        (Jsym.astype(np.float64) - Jhi16.astype(np.float64))
        .astype(np.float32).astype(ml_dtypes.bfloat16))
    J1b = np.ascontiguousarray(Jsym.astype(ml_dtypes.bfloat16))

    nc = _build(steps_i, kappa_f, has_h)
    global LAST_NC
    LAST_NC = nc

    in_maps = []
    for i in range(N_CORES):
        m = {"s": np.ascontiguousarray(s[i * B_SH:(i + 1) * B_SH]),
             "jhi": Jhi16, "jlo": Jlo16, "j1b": J1b}
        if has_h:
            m["h"] = h
        in_maps.append(m)

    trace = os.environ.get("CAM_TRACE", "") == "1"
    res = run_bass_kernel_spmd(nc, in_maps, core_ids=list(range(N_CORES)),
                               trace=trace)
    global LAST_RESULTS
    LAST_RESULTS = res
    out = np.concatenate([r["out"] for r in res.results], axis=0)
    return out.astype(np.float32, copy=False)


if __name__ == "__main__":
    rng = np.random.default_rng(0)
    s = rng.standard_normal((B, N)).astype(np.float32)
    J0 = (0.01 * rng.standard_normal((N, N))).astype(np.float32)
    J = ((J0 + J0.T) / 2).astype(np.float32)
    out = kernel(s=s, J=J, h=np.zeros(N, np.float32),
                 kappa=np.float32(0.2), steps=3)
    print(out.shape, np.unique(out, return_counts=True))
